# revision 1
# baseline (speedup 1.0000x reference)
"""CrossBandBiMamba Trainium2 kernel (8 NeuronCores, data-parallel over batch).

Layout: channels d=128 on partitions everywhere. Each core processes one batch
element = 300 independent band sequences of length K=30.

Phases (to avoid ACT table-set thrash):
  P0: LayerNorm stats (Square/Sqrt) + xn build
  P1: matmuls + all Sigmoid-set activations (silu via sigmoid+STT), spill to DRAM
  P2: Softplus + selective-scan + output matmuls
"""
import numpy as np
from contextlib import ExitStack

CH, T, K, S, NCORES = 128, 300, 30, 16, 8
TKF = T * K  # 9000

_CACHE = {}


def _host_prep(inputs):
    """Fold LN affine / conv / output combine into weights (host-side, tiny)."""
    import ml_dtypes
    bf16 = ml_dtypes.bfloat16
    f32 = np.float32
    g = inputs['ln_g'].astype(f32)
    b = inputs['ln_b'].astype(f32)
    comb_W = inputs['comb_W'].astype(f32)
    W = {}
    for d in ('f', 'b'):
        Win = inputs[d + '_Win'].astype(f32)
        bin_ = inputs[d + '_bin'].astype(f32)
        convw = inputs[d + '_convw'].astype(f32)
        convb = inputs[d + '_convb'].astype(f32)
        Wp, Wzp = Win[:, :CH], Win[:, CH:]
        wconv = np.stack([(g[:, None] * Wp) * convw[:, tau][None, :]
                          for tau in range(4)], axis=1)  # (128, 4, 128)
        W[d + '_wconv'] = wconv.reshape(CH, 4 * CH).astype(bf16)
        W[d + '_wz'] = (g[:, None] * Wzp).astype(bf16)
        bias1 = b @ Wp + bin_[:CH]
        W[d + '_bz'] = (b @ Wzp + bin_[CH:]).reshape(CH, 1).astype(f32)
        bc = np.stack([bias1 * convw[:, 3 - j:4].sum(1) + convb
                       for j in range(4)], axis=1)  # (128,4) j = #taps-1
        W[d + '_bconv'] = bc.astype(f32)
        wx = inputs[d + '_Wx'].astype(f32).copy()
        wx[:, 8:24] *= -1.0  # u = (ln(E)*xp)*(-B) since ln(E) = -dt
        W[d + '_wx'] = wx.astype(bf16)
        W[d + '_wdt'] = inputs[d + '_Wdt'].astype(f32).astype(bf16)
        bdt = inputs[d + '_bdt'].astype(f32)
        W[d + '_bdt'] = bdt.reshape(CH, 1)
        W[d + '_nbdt'] = (-bdt).reshape(CH, 1)
        W[d + '_dp'] = inputs[d + '_D'].astype(f32).reshape(CH, 1)
        half = comb_W[:CH] if d == 'f' else comb_W[CH:]
        W[d + '_woc'] = (inputs[d + '_Wout'].astype(f32) @ half).astype(bf16)
    bias_comb = (inputs['comb_b'].astype(f32)
                 + inputs['f_bout'].astype(f32) @ comb_W[:CH]
                 + inputs['b_bout'].astype(f32) @ comb_W[CH:])
    W['bias_comb'] = bias_comb.reshape(CH, 1).astype(f32)
    return W


def _chunks():
    # t-chunks: 17 chunks of 17 seqs + 1 of 11 (300 total); FD = tc*30 <= 510
    out, t0 = [], 0
    for tc in [17] * 17 + [11]:
        out.append((t0, tc))
        t0 += tc
    return out


def _build():
    import os
    SKIP = set(os.environ.get('KSKIP', '').split(','))
    import concourse.bass as bass
    import concourse.tile as tile
    from concourse import mybir, bacc

    f32 = mybir.dt.float32
    bf = mybir.dt.bfloat16
    AF = mybir.ActivationFunctionType
    OP = mybir.AluOpType

    nc = bacc.Bacc("TRN2", target_bir_lowering=False, debug=False,
                   num_devices=NCORES)

    x_d = nc.dram_tensor("x", [CH, TKF], f32, kind="ExternalInput")
    o_d = nc.dram_tensor("out", [CH, TKF], f32, kind="ExternalOutput")
    wd = {}
    for d in ('f', 'b'):
        wd[d + '_wconv'] = nc.dram_tensor(d + "_wconv", [CH, 4 * CH], bf, kind="ExternalInput")
        wd[d + '_wz'] = nc.dram_tensor(d + "_wz", [CH, CH], bf, kind="ExternalInput")
        wd[d + '_wx'] = nc.dram_tensor(d + "_wx", [CH, 40], bf, kind="ExternalInput")
        wd[d + '_wdt'] = nc.dram_tensor(d + "_wdt", [8, CH], bf, kind="ExternalInput")
        wd[d + '_woc'] = nc.dram_tensor(d + "_woc", [CH, CH], bf, kind="ExternalInput")
        wd[d + '_bconv'] = nc.dram_tensor(d + "_bconv", [CH, 4], f32, kind="ExternalInput")
        for n in ('bz', 'bdt', 'nbdt', 'dp'):
            wd[f'{d}_{n}'] = nc.dram_tensor(f"{d}_{n}", [CH, 1], f32, kind="ExternalInput")
    wd['bias_comb'] = nc.dram_tensor("bias_comb", [CH, 1], f32, kind="ExternalInput")

    # spill DRAM (internal)
    stats_dram = nc.dram_tensor("sp_stats", [2, TKF], mybir.dt.float32)
    xn_dram = nc.dram_tensor("sp_xn", [CH, TKF], mybir.dt.bfloat16)
    lnrow_dram = nc.dram_tensor("sp_lnrow", [2, TKF], mybir.dt.float32)
    sp = {}
    for d in ('f', 'b'):
        for n, shp, dt in (('xp', [CH, TKF], bf), ('zs', [CH, TKF], bf),
                           ('E', [CH, TKF], bf),
                           ('bc', [32, TKF], bf)):
            sp[f'{d}_{n}'] = nc.dram_tensor(f"sp_{d}_{n}", shp, dt)

    CHK = _chunks()

    with tile.TileContext(nc) as tc_, ExitStack() as ctx:
        tcx = tc_
        wpool = ctx.enter_context(tcx.tile_pool(name="w", bufs=1))
        big = ctx.enter_context(tcx.tile_pool(name="big", bufs=1))
        sbuf = ctx.enter_context(tcx.tile_pool(name="sb", bufs=2))
        scan_p = ctx.enter_context(tcx.tile_pool(name="scan", bufs=1))
        bcp = ctx.enter_context(tcx.tile_pool(name="bcp", bufs=1))
        psA = ctx.enter_context(tcx.tile_pool(name="psA", bufs=1, space="PSUM"))   # 1-bank tiles
        psB = ctx.enter_context(tcx.tile_pool(name="psB", bufs=2, space="PSUM"))
        psO = ctx.enter_context(tcx.tile_pool(name="psO", bufs=2, space="PSUM"))

        # ---- load weights ----
        wt = {}
        for name, dten in wd.items():
            shp = list(dten.shape)
            t = wpool.tile(shp, dten.dtype, name="w_" + name)
            nc.sync.dma_start(t[:], dten[:, :])
            wt[name] = t
        ones_c = wpool.tile([CH, 1], f32, name="ones_c")
        nc.vector.memset(ones_c[:], 1.0 / CH)
        ones_r = wpool.tile([33, CH], bf, name="ones_r")
        nc.vector.memset(ones_r[:], 1.0)
        ones_rf = wpool.tile([1, CH], f32, name="ones_rf")
        nc.vector.memset(ones_rf[:], 1.0)


        # ================= P0: LN stats =================
        for (t0, tcn) in CHK:
            c0, cn = t0 * K, tcn * K
            xch_t = sbuf.tile([CH, 510], f32, tag="xf32")
            nc.sync.dma_start(xch_t[:, :cn], x_d[:, c0:c0 + cn])
            xch = xch_t[:, :cn]
            sq = sbuf.tile([CH, 510], f32, tag="scr_f32")
            nc.scalar.square(sq[:, :cn], xch)
            st_mu = psA.tile([1, 510], f32, tag="psX")
            nc.tensor.matmul(st_mu[0:1, :cn], ones_c[:], xch, start=True, stop=True)
            st_m2 = psA.tile([1, 510], f32, tag="psY")
            nc.tensor.matmul(st_m2[0:1, :cn], ones_c[:], sq[:, :cn], start=True, stop=True)
            st_sb = sbuf.tile([1, 1020], f32, tag="p0st")
            nc.scalar.copy(st_sb[0:1, 0:cn], st_mu[0:1, :cn])
            nc.scalar.copy(st_sb[0:1, 510:510 + cn], st_m2[0:1, :cn])
            nc.sync.dma_start(stats_dram[0:1, c0:c0 + cn], st_sb[0:1, 0:cn])
            nc.sync.dma_start(stats_dram[1:2, c0:c0 + cn], st_sb[0:1, 510:510 + cn])
        # reshape 9000 -> (90,100) via DMA, do math, back
        s2 = big.tile([90, 200], f32, name="s2")
        nc.sync.dma_start(s2[:, 0:100], stats_dram[0:1, :])    # mu
        nc.sync.dma_start(s2[:, 100:200], stats_dram[1:2, :])  # msq
        mu2 = sbuf.tile([90, 100], f32, tag="mu2")
        nc.scalar.square(mu2[:], s2[:, 0:100])
        var = sbuf.tile([90, 100], f32, tag="var")
        nc.vector.tensor_sub(var[:], s2[:, 100:200], mu2[:])
        eps_t = wpool.tile([90, 1], f32, name="eps_t")
        nc.vector.memset(eps_t[:], 1e-5)
        std = sbuf.tile([90, 100], f32, tag="std")
        nc.scalar.activation(std[:], var[:], AF.Sqrt, bias=eps_t[:, 0:1], scale=1.0)
        rstd = sbuf.tile([90, 100], f32, tag="rstd")
        nc.vector.reciprocal(rstd[:], std[:])
        nmr = sbuf.tile([90, 100], f32, tag="nmr")
        nc.vector.scalar_tensor_tensor(nmr[:], s2[:, 0:100], -1.0, rstd[:],
                                       OP.mult, OP.mult)
        nc.sync.dma_start(lnrow_dram[0:1, :], rstd[:])
        nc.sync.dma_start(lnrow_dram[1:2, :], nmr[:])

        # xn = x*rstd + (-mu*rstd)   (broadcast rows via PE)
        for (t0, tcn) in CHK:
            c0, cn = t0 * K, tcn * K
            lnr = sbuf.tile([1, 1020], f32, tag="lnr")
            nc.sync.dma_start(lnr[0:1, 0:cn], lnrow_dram[0:1, c0:c0 + cn])
            nc.sync.dma_start(lnr[0:1, 510:510 + cn], lnrow_dram[1:2, c0:c0 + cn])
            lx = sbuf.tile([CH, 510], f32, tag="xf32")
            nc.sync.dma_start(lx[:, :cn], x_d[:, c0:c0 + cn])
            rb = psA.tile([CH, 510], f32, tag="psX")
            nc.tensor.matmul(rb[:, :cn], ones_rf[:], lnr[0:1, 0:cn],
                             start=True, stop=True)
            nm = psA.tile([CH, 510], f32, tag="psY")
            nc.tensor.matmul(nm[:, :cn], ones_rf[:], lnr[0:1, 510:510 + cn],
                             start=True, stop=True)
            t0s = sbuf.tile([CH, 510], f32, tag="scr_f32")
            nc.vector.tensor_mul(t0s[:, :cn], lx[:, :cn], rb[:, :cn])
            xnc = sbuf.tile([CH, 510], bf, tag="xnc")
            nc.vector.tensor_add(xnc[:, :cn], t0s[:, :cn], nm[:, :cn])
            nc.sync.dma_start(xn_dram[:, c0:c0 + cn], xnc[:, :cn])

        # ================= P1: matmuls + sigmoid set =================
        for d in ('f', 'b'):
            fwd = (d == 'f')
            for (t0, tcn) in CHK:
                c0, cn = t0 * K, tcn * K
                xn_t = sbuf.tile([CH, 510], bf, tag="xnc")
                nc.sync.dma_start(xn_t[:, :cn], xn_dram[:, c0:c0 + cn])
                xn3 = xn_t[:, :cn].rearrange("p (t k) -> p t k", k=K)
                xp_ps = psA.tile([CH, 510], f32, tag="psX")
                xp3 = xp_ps[:, :cn].rearrange("p (t k) -> p t k", k=K)
                wc = wt[d + '_wconv'][:].rearrange("p (t c) -> p t c", c=CH)
                # tau=3 (shift 0) first, full range, start=True
                xnv = xn3 if fwd else xn3[:, :, ::-1]
                for i, tau in enumerate((3, 2, 1, 0)):
                    sh = 3 - tau
                    rhs = xnv[:, :, 0:K - sh]
                    out = xp3[:, :, sh:K] if sh else xp3[:, :, :]
                    nc.tensor.matmul(out, wc[:, tau:tau + 1, :], rhs,
                                     start=(i == 0), stop=(i == 3))
                z_ps = psA.tile([CH, 510], f32, tag="psY")
                nc.tensor.matmul(z_ps[:, :cn].rearrange("p (t k) -> p t k", k=K),
                                 wt[d + '_wz'][:], xnv,
                                 start=True, stop=True)
                # silu on conv output, k-grouped bias
                xp_sb = sbuf.tile([CH, 510], bf, tag="xp_sb")
                xps3 = xp_sb[:, :cn].rearrange("p (t k) -> p t k", k=K)
                sg = sbuf.tile([CH, 510], bf, tag="sg")
                sg3 = sg[:, :cn].rearrange("p (t k) -> p t k", k=K)
                groups = [(0, 1, 0), (1, 2, 1), (2, 3, 2), (3, K, 3)]
                for (ka, kb, j) in groups:
                    bc = wt[d + '_bconv'][:, j:j + 1]
                    nc.scalar.activation(sg3[:, :, ka:kb], xp3[:, :, ka:kb],
                                         AF.Sigmoid, bias=bc, scale=1.0)
                    nc.vector.scalar_tensor_tensor(
                        xps3[:, :, ka:kb], xp3[:, :, ka:kb], bc, sg3[:, :, ka:kb],
                        OP.add, OP.mult)
                # silu on z
                zs_sb = sbuf.tile([CH, 510], bf, tag="zs_sb")
                sgz = sbuf.tile([CH, 510], bf, tag="sgz")
                nc.scalar.activation(sgz[:, :cn], z_ps[:, :cn], AF.Sigmoid,
                                     bias=wt[d + '_bz'][:, 0:1], scale=1.0)
                nc.vector.scalar_tensor_tensor(
                    zs_sb[:, :cn], z_ps[:, :cn], wt[d + '_bz'][:, 0:1],
                    sgz[:, :cn], OP.add, OP.mult)
                # xdbl = xp @ Wx
                xd_ps = psA.tile([40, 510], f32, tag="psX")
                nc.tensor.matmul(xd_ps[:, :cn], wt[d + '_wx'][:], xp_sb[:, :cn],
                                 start=True, stop=True)
                xd_sb = sbuf.tile([40, 510], bf, tag="xd_sb")
                nc.scalar.copy(xd_sb[:, :cn], xd_ps[:, :cn])
                # dt_raw = dt_r @ Wdt
                dtr_ps = psA.tile([CH, 510], f32, tag="psX")
                nc.tensor.matmul(dtr_ps[:, :cn], wt[d + '_wdt'][:],
                                 xd_sb[0:8, :cn], start=True, stop=True)
                E_sb = sbuf.tile([CH, 510], bf, tag="E_sb")
                nc.scalar.activation(E_sb[:, :cn], dtr_ps[:, :cn], AF.Sigmoid,
                                     bias=wt[d + '_nbdt'][:, 0:1], scale=-1.0)
                # spills
                nc.sync.dma_start(sp[d + '_xp'][:, c0:c0 + cn], xp_sb[:, :cn])
                nc.sync.dma_start(sp[d + '_zs'][:, c0:c0 + cn], zs_sb[:, :cn])
                nc.sync.dma_start(sp[d + '_E'][:, c0:c0 + cn], E_sb[:, :cn])
                nc.sync.dma_start(sp[d + '_bc'][:, c0:c0 + cn], xd_sb[8:40, :cn])

        # ================= P2: softplus + scan =================
        for (t0, tcn) in CHK:
            c0, cn = t0 * K, tcn * K
            yg = {}
            DD = {}
            for d in ('f', 'b'):
                xp_sb = sbuf.tile([CH, 510], bf, tag="r_xp_" + d)
                nc.sync.dma_start(xp_sb[:, :cn], sp[d + '_xp'][:, c0:c0 + cn])
                zs_sb = sbuf.tile([CH, 510], bf, tag="r_zs_" + d)
                nc.sync.dma_start(zs_sb[:, :cn], sp[d + '_zs'][:, c0:c0 + cn])
                E_sb = sbuf.tile([CH, 510], bf, tag="r_E")
                nc.sync.dma_start(E_sb[:, :cn], sp[d + '_E'][:, c0:c0 + cn])

                dt_sb = sbuf.tile([CH, 510], bf, tag="dt_sb")
                nc.scalar.activation(dt_sb[:, :cn], E_sb[:, :cn], AF.Ln)
                v_sb = sbuf.tile([CH, 510], bf, tag="v_sb")
                nc.vector.tensor_mul(v_sb[:, :cn], dt_sb[:, :cn], xp_sb[:, :cn])
                v3 = v_sb[:, :cn].rearrange("p (t k) -> p t k", k=K)
                E3 = E_sb[:, :cn].rearrange("p (t k) -> p t k", k=K)

                dA = scan_p.tile([CH, 510 * S], bf, tag="dA", bufs=2)
                dA4 = dA[:, :cn * S].rearrange("p (t k s) -> p t k s", k=K, s=S)
                dt3 = dt_sb[:, :cn].rearrange("p (t k) -> p t k", k=K)
                nc.vector.tensor_copy(dA4[:, :, :, 0:1], E3)
                for s in range(1, S):
                    nc.scalar.activation(dA4[:, :, :, s:s + 1], dt3, AF.Exp,
                                         scale=float(s + 1))
                u = scan_p.tile([CH, 510 * S], bf, tag="u", bufs=2)
                u4 = u[:, :cn * S].rearrange("p (t k s) -> p t k s", k=K, s=S)
                C_t = scan_p.tile([CH, 510 * S], bf, tag="C_t", bufs=2)
                C4 = C_t[:, :cn * S].rearrange("p (t k s) -> p t k s", k=K, s=S)
                for q in range(8):
                    bcq = bcp.tile([1, 2 * 510], bf, tag="bcq", bufs=3)
                    nc.sync.dma_start(bcq[0:1, :2 * cn],
                                      sp[d + '_bc'][2 * q:2 * q + 2, c0:c0 + cn])
                    bq = psB.tile([CH, 2, 512], f32, tag="bq")
                    for j in range(2):
                        nc.tensor.matmul(bq[:, j, :cn], ones_r[0:1, :],
                                         bcq[0:1, j * cn:(j + 1) * cn],
                                         start=True, stop=True)
                    for j in range(2):
                        s = 2 * q + j
                        nc.vector.tensor_tensor(
                            u4[:, :, :, s:s + 1],
                            bq[:, j, :cn], v3, OP.mult)
                for s in range(S):
                    ccs = bcp.tile([1, 510], bf, tag="ccs", bufs=2)
                    nc.sync.dma_start(ccs[0:1, :cn],
                                      sp[d + '_bc'][16 + s:17 + s, c0:c0 + cn])
                    nc.gpsimd.partition_broadcast(
                        C4[:, :, :, s:s + 1], ccs[0:1, :cn])

                H = scan_p.tile([CH, 510 * S], bf, tag="H", bufs=2)
                H4 = H[:, :cn * S].rearrange("p (t k s) -> p t k s", k=K, s=S)
                DD[d] = (dA4, u4, C4, H4, xp_sb, zs_sb)
            # interleaved scans (two independent chains hide RAW latency)
            dAf, uf, Cf, Hf, _, _ = DD['f']
            dAb, ub, Cb, Hb, _, _ = DD['b']
            for k in range(K):
                if k == 0:
                    nc.vector.tensor_copy(Hf[:, :, 0, :], uf[:, :, 0, :])
                    nc.vector.tensor_copy(Hb[:, :, 0, :], ub[:, :, 0, :])
                else:
                    nc.vector.tensor_tensor(Hf[:, :, k, :], Hf[:, :, k - 1, :],
                                            dAf[:, :, k, :], OP.mult)
                    nc.vector.tensor_tensor(Hb[:, :, k, :], Hb[:, :, k - 1, :],
                                            dAb[:, :, k, :], OP.mult)
                    nc.vector.tensor_tensor(Hf[:, :, k, :], Hf[:, :, k, :],
                                            uf[:, :, k, :], OP.add)
                    nc.vector.tensor_tensor(Hb[:, :, k, :], Hb[:, :, k, :],
                                            ub[:, :, k, :], OP.add)
            for d in ('f', 'b'):
                dA4, u4, C4, H4, xp_sb, zs_sb = DD[d]
                y_sb = sbuf.tile([CH, 510], bf, tag="y_sb")
                y3 = y_sb[:, :cn].rearrange("p (t k) -> p t k", k=K)
                nc.vector.tensor_tensor(H4[:], H4[:], C4[:], OP.mult)
                nc.vector.tensor_tensor(H4[:, :, :, 0:8], H4[:, :, :, 0:8],
                                        H4[:, :, :, 8:16], OP.add)
                nc.vector.tensor_tensor(H4[:, :, :, 0:4], H4[:, :, :, 0:4],
                                        H4[:, :, :, 4:8], OP.add)
                nc.vector.tensor_tensor(H4[:, :, :, 0:2], H4[:, :, :, 0:2],
                                        H4[:, :, :, 2:4], OP.add)
                nc.vector.tensor_tensor(y3[:], H4[:, :, :, 0:1], H4[:, :, :, 1:2],
                                        OP.add)
                t1 = sbuf.tile([CH, 510], bf, tag="t1")
                nc.vector.scalar_tensor_tensor(t1[:, :cn], xp_sb[:, :cn],
                                               wt[d + '_dp'][:, 0:1], y_sb[:, :cn],
                                               OP.mult, OP.add)
                ygt = sbuf.tile([CH, 510], bf, tag="yg_" + d)
                nc.vector.tensor_mul(ygt[:, :cn], t1[:, :cn], zs_sb[:, :cn])
                yg[d] = ygt
            # output: out = ygf @ woc_f + rev(ygb) @ woc_b + bias_comb + x
            o_ps = psO.tile([CH, 510], f32, tag="o_ps")
            nc.tensor.matmul(o_ps[:, :cn], wt['f_woc'][:], yg['f'][:, :cn],
                             start=True, stop=False)
            ygb3 = yg['b'][:, :cn].rearrange("p (t k) -> p t k", k=K)
            nc.tensor.matmul(o_ps[:, :cn].rearrange("p (t k) -> p t k", k=K),
                             wt['b_woc'][:], ygb3[:, :, ::-1],
                             start=False, stop=True)
            x2 = sbuf.tile([CH, 510], f32, tag="xf32")
            nc.sync.dma_start(x2[:, :cn], x_d[:, c0:c0 + cn])
            o_sb = sbuf.tile([CH, 510], f32, tag="scr_f32")
            nc.vector.scalar_tensor_tensor(o_sb[:, :cn], o_ps[:, :cn],
                                           wt['bias_comb'][:, 0:1],
                                           x2[:, :cn], OP.add, OP.add)
            nc.sync.dma_start(o_d[:, c0:c0 + cn], o_sb[:, :cn])

    nc.compile()
    return nc


def kernel(**inputs):
    from concourse.bass_utils import run_bass_kernel_spmd

    if 'nc' not in _CACHE:
        _CACHE['nc'] = _build()
    nc = _CACHE['nc']
    W = _host_prep(inputs)
    x = np.asarray(inputs['x'], dtype=np.float32)  # (8, 128, 300, 30)
    in_maps = []
    for i in range(NCORES):
        m = dict(W)
        m['x'] = np.ascontiguousarray(x[i].reshape(CH, TKF))
        in_maps.append(m)
    res = run_bass_kernel_spmd(nc, in_maps, core_ids=list(range(NCORES)))
    out = np.stack([np.asarray(res.results[i]['out'], dtype=np.float32)
                    .reshape(CH, T, K) for i in range(NCORES)])
    return out



# revision 7
# speedup vs baseline: 1.9152x; 1.9152x over previous
"""CrossBandBiMamba Trainium2 kernel (8 NeuronCores, data-parallel over batch).

v2: fully-fused single pass per t-chunk, fp16 compute, hardware scan op.

Layout: channels d=128 on partitions; free dim (t, k) with k innermost.
Per (dir, chunk) unit the selective-scan state is expanded s-major:
[128, S=16, T_c, K=30] fp16, scanned by ONE tensor_tensor_scan whose
cross-sequence chaining is neutralized by dA=0 at each k=0 slot.
B/C rows are broadcast to 128 partitions via single multi-row DMA reads
from a DRAM bounce buffer. H overwrites dA in place; C~ reuses u's buffer.
"""
import numpy as np
from contextlib import ExitStack

CH, T, K, S, NCORES = 128, 300, 30, 16, 8
TKF = T * K  # 9000
TC = 34      # t-chunk; 8 full chunks of 34 + tail of 28
NB = S * TC * K
DA_ACT_FROM = 8  # dA slabs s>=this via ACT exp; below via DVE power doubling

_CACHE = {}


def _chunks():
    out, t0 = [], 0
    while t0 < T:
        tc = min(TC, T - t0)
        out.append((t0, tc))
        t0 += tc
    return out


def _tsubs(tcn):
    out, a = [], 0
    while a < tcn:
        b = min(tcn, a + 17)
        out.append((a, b))
        a = b
    return out


def _host_prep(inputs):
    """Fold LN affine / conv / output combine into weights (host-side, tiny)."""
    f16 = np.float16
    f32 = np.float32
    g = inputs['ln_g'].astype(f32)
    b = inputs['ln_b'].astype(f32)
    comb_W = inputs['comb_W'].astype(f32)
    W = {}
    for d in ('f', 'b'):
        Win = inputs[d + '_Win'].astype(f32)
        bin_ = inputs[d + '_bin'].astype(f32)
        convw = inputs[d + '_convw'].astype(f32)
        convb = inputs[d + '_convb'].astype(f32)
        Wp, Wzp = Win[:, :CH], Win[:, CH:]
        wconv = np.stack([(g[:, None] * Wp) * convw[:, tau][None, :]
                          for tau in range(4)], axis=1)  # (128, 4, 128)
        W[d + '_wconv'] = wconv.reshape(CH, 4 * CH).astype(f16)
        W[d + '_wz'] = (g[:, None] * Wzp).astype(f16)
        bias1 = b @ Wp + bin_[:CH]
        W[d + '_bz'] = (b @ Wzp + bin_[CH:]).reshape(1, CH).astype(f16)
        bc = np.stack([bias1 * convw[:, 3 - j:4].sum(1) + convb
                       for j in range(4)], axis=0)  # (4, 128); j = #taps-1
        W[d + '_bconv'] = bc.reshape(1, 4 * CH).astype(f16)
        wx = inputs[d + '_Wx'].astype(f32).copy()
        wx[:, 8:24] *= -1.0  # u = (ln(E)*xp)*(-B) since ln(E) = -dt
        W[d + '_wx'] = wx.astype(f16)
        W[d + '_wdt'] = inputs[d + '_Wdt'].astype(f32).astype(f16)
        bdt = inputs[d + '_bdt'].astype(f32)
        W[d + '_nbdt'] = (-bdt).reshape(CH, 1)
        W[d + '_dp'] = inputs[d + '_D'].astype(f32).reshape(CH, 1)
        half = comb_W[:CH] if d == 'f' else comb_W[CH:]
        W[d + '_woc'] = (inputs[d + '_Wout'].astype(f32) @ half).astype(f16)
    bias_comb = (inputs['comb_b'].astype(f32)
                 + inputs['f_bout'].astype(f32) @ comb_W[:CH]
                 + inputs['b_bout'].astype(f32) @ comb_W[CH:])
    W['bias_comb'] = bias_comb.reshape(1, CH).astype(f16)
    W['ident'] = np.eye(CH, dtype=f16)
    return W


def _build():
    import concourse.bass as bass
    import concourse.tile as tile
    from concourse import mybir, bacc

    f32 = mybir.dt.float32
    f16 = mybir.dt.float16
    AF = mybir.ActivationFunctionType
    OP = mybir.AluOpType

    nc = bacc.Bacc("TRN2", target_bir_lowering=False, debug=False,
                   num_devices=NCORES)

    x_d = nc.dram_tensor("x", [CH, TKF], f32, kind="ExternalInput")
    o_d = nc.dram_tensor("out", [CH, TKF], f32, kind="ExternalOutput")
    wd = {}
    for d in ('f', 'b'):
        wd[d + '_wconv'] = nc.dram_tensor(d + "_wconv", [CH, 4 * CH], f16, kind="ExternalInput")
        wd[d + '_wz'] = nc.dram_tensor(d + "_wz", [CH, CH], f16, kind="ExternalInput")
        wd[d + '_wx'] = nc.dram_tensor(d + "_wx", [CH, 40], f16, kind="ExternalInput")
        wd[d + '_wdt'] = nc.dram_tensor(d + "_wdt", [8, CH], f16, kind="ExternalInput")
        wd[d + '_woc'] = nc.dram_tensor(d + "_woc", [CH, CH], f16, kind="ExternalInput")
        wd[d + '_bconv'] = nc.dram_tensor(d + "_bconv", [1, 4 * CH], f16, kind="ExternalInput")
        wd[d + '_bz'] = nc.dram_tensor(d + "_bz", [1, CH], f16, kind="ExternalInput")
        wd[d + '_nbdt'] = nc.dram_tensor(d + "_nbdt", [CH, 1], f32, kind="ExternalInput")
        wd[d + '_dp'] = nc.dram_tensor(d + "_dp", [CH, 1], f32, kind="ExternalInput")
    wd['bias_comb'] = nc.dram_tensor("bias_comb", [1, CH], f16, kind="ExternalInput")
    wd['ident'] = nc.dram_tensor("ident", [CH, CH], f16, kind="ExternalInput")

    stats_dram = nc.dram_tensor("sp_stats", [2, TKF], f32)
    lnrow_dram = nc.dram_tensor("sp_lnrow", [2, TKF], f16)
    bcrow_dram = {d: nc.dram_tensor(f"sp_bc_{d}", [32, TKF], f16)
                  for d in ('f', 'b')}

    CHK = _chunks()

    with tile.TileContext(nc) as tc_, ExitStack() as ctx:
        wpool = ctx.enter_context(tc_.tile_pool(name="w", bufs=1))
        bigA = ctx.enter_context(tc_.tile_pool(name="bigA", bufs=2))
        bigU = ctx.enter_context(tc_.tile_pool(name="bigU", bufs=1))
        bigB = ctx.enter_context(tc_.tile_pool(name="bigB", bufs=1))
        sb1 = ctx.enter_context(tc_.tile_pool(name="sb1", bufs=1))
        sb2 = ctx.enter_context(tc_.tile_pool(name="sb2", bufs=2))
        psm = ctx.enter_context(tc_.tile_pool(name="psm", bufs=2, space="PSUM"))
        pso = ctx.enter_context(tc_.tile_pool(name="pso", bufs=2, space="PSUM"))

        wt = {}
        for name, dten in wd.items():
            t = wpool.tile(list(dten.shape), dten.dtype, name="w_" + name)
            nc.sync.dma_start(t[:], dten[:, :])
            wt[name] = t
        ones_c = wpool.tile([CH, 1], f16, name="ones_c")
        nc.vector.memset(ones_c[:], 1.0 / CH)
        ones_r = wpool.tile([1, 512], f16, name="ones_r")
        nc.vector.memset(ones_r[:], 1.0)
        eps_t = wpool.tile([90, 1], f32, name="eps_t")
        nc.vector.memset(eps_t[:], 1e-5)

        # ================= phase 0: LN stats =================
        for (t0, tcn) in CHK:
            c0, cn = t0 * K, tcn * K
            xbf = sb1.tile([CH, TC * K], f16, tag="p0x")
            nc.gpsimd.dma_start(xbf[:, :cn], x_d[:, c0:c0 + cn])
            sq = sb1.tile([CH, TC * K], f16, tag="p0sq")
            nc.scalar.activation(sq[:, :cn], xbf[:, :cn], AF.Square)
            st = psm.tile([CH, 2, 512], f32, tag="ps")
            st2 = psm.tile([CH, 2, 512], f32, tag="ps")
            nsub = (cn + 511) // 512
            for j in range(nsub):
                a, b = j * 512, min(cn, (j + 1) * 512)
                nc.tensor.matmul(st[0:1, j, :b - a], ones_c[:], xbf[:, a:b],
                                 start=True, stop=True)
                nc.tensor.matmul(st2[0:1, j, :b - a], ones_c[:], sq[:, a:b],
                                 start=True, stop=True)
            strow = sb1.tile([1, 2 * TC * K], f32, tag="p0r")
            for j in range(nsub):
                a, b = j * 512, min(cn, (j + 1) * 512)
                nc.scalar.copy(strow[0:1, a:b], st[0:1, j, :b - a])
                nc.scalar.copy(strow[0:1, TC * K + a:TC * K + b],
                               st2[0:1, j, :b - a])
            nc.sync.dma_start(stats_dram[0:1, c0:c0 + cn], strow[0:1, :cn])
            nc.sync.dma_start(stats_dram[1:2, c0:c0 + cn],
                              strow[0:1, TC * K:TC * K + cn])
        s2 = sb1.tile([90, 200], f32, tag="p0s2")
        nc.sync.dma_start(s2[:, 0:100], stats_dram[0:1, :])
        nc.sync.dma_start(s2[:, 100:200], stats_dram[1:2, :])
        mu2 = sb1.tile([90, 100], f32, tag="p0m2")
        nc.scalar.square(mu2[:], s2[:, 0:100])
        var = sb1.tile([90, 100], f32, tag="p0v")
        nc.vector.tensor_sub(var[:], s2[:, 100:200], mu2[:])
        std = sb1.tile([90, 100], f32, tag="p0sd")
        nc.scalar.activation(std[:], var[:], AF.Sqrt, bias=eps_t[:, 0:1],
                             scale=1.0)
        rstd = sb1.tile([90, 100], f32, tag="p0rs")
        nc.vector.reciprocal(rstd[:], std[:])
        lnr = sb1.tile([90, 200], f16, tag="p0ln")
        nc.vector.tensor_copy(lnr[:, 0:100], rstd[:])
        nc.vector.tensor_mul(lnr[:, 100:200], s2[:, 0:100], rstd[:])
        nc.sync.dma_start(lnrow_dram[0:1, :], lnr[:, 0:100])
        nc.sync.dma_start(lnrow_dram[1:2, :], lnr[:, 100:200])

        # ================= main: per (chunk, dir) =================
        for (t0, tcn) in CHK:
            c0, cn = t0 * K, tcn * K
            nb = S * cn
            tsubs = _tsubs(tcn)
            xbf = sb2.tile([CH, TC * K], f16, tag="xbf")
            nc.gpsimd.dma_start(xbf[:, :cn], x_d[:, c0:c0 + cn])
            lnb = sb2.tile([CH, 2, TC * K], f16, tag="lnb")
            nc.sync.dma_start(lnb[:, :, :cn],
                              lnrow_dram[:, c0:c0 + cn].partition_broadcast(CH))
            xn = sb1.tile([CH, TC * K], f16, tag="xn")
            t0s = sb1.tile([CH, TC * K], f16, tag="t0s")
            nc.vector.tensor_mul(t0s[:, :cn], xbf[:, :cn], lnb[:, 0, :cn])
            nc.vector.tensor_sub(xn[:, :cn], t0s[:, :cn], lnb[:, 1, :cn])
            yg_f = None
            for d in ('f', 'b'):
                fwd = (d == 'f')
                xn3 = xn[:, :cn].rearrange("p (t k) -> p t k", k=K)
                xnv = xn3 if fwd else xn3[:, :, ::-1]
                conv_ps = psm.tile([CH, 2, 512], f32, tag="ps")
                z_ps = psm.tile([CH, 2, 512], f32, tag="ps")
                wc = wt[d + '_wconv'][:].rearrange("p (t c) -> p t c", c=CH)
                for j, (ta, tb) in enumerate(tsubs):
                    w = (tb - ta) * K
                    cp3 = conv_ps[:, j, :w].rearrange("p (t k) -> p t k", k=K)
                    for (ka, kb, jj) in ((0, 1, 0), (1, 2, 1), (2, 3, 2), (3, K, 3)):
                        nc.tensor.matmul(cp3[:, :, ka:kb],
                                         wt[d + '_bconv'][0:1, jj * CH:(jj + 1) * CH],
                                         ones_r[0:1, 0:(tb - ta) * (kb - ka)]
                                         .rearrange("p (t k) -> p t k", k=kb - ka),
                                         start=True, stop=False)
                    xsub = xnv[:, ta:tb, :]
                    for i, tau in enumerate((3, 2, 1, 0)):
                        sh = 3 - tau
                        rhs = xsub[:, :, 0:K - sh]
                        outv = cp3[:, :, sh:K] if sh else cp3[:, :, :]
                        nc.tensor.matmul(outv, wc[:, tau:tau + 1, :], rhs,
                                         start=False, stop=(i == 3))
                    zp3 = z_ps[:, j, :w].rearrange("p (t k) -> p t k", k=K)
                    nc.tensor.matmul(zp3, wt[d + '_bz'][0:1, :],
                                     ones_r[0:1, 0:w]
                                     .rearrange("p (t k) -> p t k", k=K),
                                     start=True, stop=False)
                    nc.tensor.matmul(zp3, wt[d + '_wz'][:], xsub,
                                     start=False, stop=True)
                xp = sb1.tile([CH, TC * K], f16, tag="xp_" + d)
                zs = sb1.tile([CH, TC * K], f16, tag="zs_" + d)
                sg = sb1.tile([CH, TC * K], f16, tag="sg")
                pc = sb1.tile([CH, TC * K], f16, tag="pc")
                for j, (ta, tb) in enumerate(tsubs):
                    a, w = ta * K, (tb - ta) * K
                    nc.scalar.activation(sg[:, a:a + w], conv_ps[:, j, :w],
                                         AF.Sigmoid)
                    nc.scalar.copy(pc[:, a:a + w], conv_ps[:, j, :w])
                nc.vector.tensor_mul(xp[:, :cn], pc[:, :cn], sg[:, :cn])
                sgz = sb1.tile([CH, TC * K], f16, tag="sgz")
                pz = sb1.tile([CH, TC * K], f16, tag="pz")
                for j, (ta, tb) in enumerate(tsubs):
                    a, w = ta * K, (tb - ta) * K
                    nc.scalar.activation(sgz[:, a:a + w], z_ps[:, j, :w],
                                         AF.Sigmoid)
                    nc.scalar.copy(pz[:, a:a + w], z_ps[:, j, :w])
                nc.vector.tensor_mul(zs[:, :cn], pz[:, :cn], sgz[:, :cn])
                xd_ps = psm.tile([CH, 2, 512], f32, tag="ps")
                for j, (ta, tb) in enumerate(tsubs):
                    a, w = ta * K, (tb - ta) * K
                    nc.tensor.matmul(xd_ps[0:40, j, :w], wt[d + '_wx'][:],
                                     xp[:, a:a + w], start=True, stop=True)
                xd_sb = sb1.tile([40, TC * K], f16, tag="xd_sb")
                for j, (ta, tb) in enumerate(tsubs):
                    a, w = ta * K, (tb - ta) * K
                    nc.scalar.copy(xd_sb[:, a:a + w], xd_ps[0:40, j, :w])
                nc.sync.dma_start(bcrow_dram[d][:, c0:c0 + cn], xd_sb[8:40, :cn])
                dtr_ps = psm.tile([CH, 2, 512], f32, tag="ps")
                for j, (ta, tb) in enumerate(tsubs):
                    a, w = ta * K, (tb - ta) * K
                    nc.tensor.matmul(dtr_ps[:, j, :w], wt[d + '_wdt'][:],
                                     xd_sb[0:8, a:a + w], start=True, stop=True)
                dAH = bigA.tile([CH, NB], f16, tag="dAH")
                for j, (ta, tb) in enumerate(tsubs):
                    a, w = ta * K, (tb - ta) * K
                    nc.scalar.activation(dAH[:, a:a + w], dtr_ps[:, j, :w],
                                         AF.Sigmoid, bias=wt[d + '_nbdt'][:, 0:1],
                                         scale=-1.0)
                dtn = sb1.tile([CH, TC * K], f16, tag="dtn")
                nc.scalar.activation(dtn[:, :cn], dAH[:, :cn], AF.Ln)
                v = sb1.tile([CH, TC * K], f16, tag="v")
                nc.vector.tensor_mul(v[:, :cn], dtn[:, :cn], xp[:, :cn])
                # dA powers by doubling; slab(i) = E^(i+1)
                m = 1
                while m < DA_ACT_FROM:
                    n2 = min(m, DA_ACT_FROM - m)
                    em = (dAH[:, (m - 1) * cn:m * cn]
                          .unsqueeze(1).broadcast_to([CH, n2, cn]))
                    lo = (dAH[:, 0:n2 * cn]
                          .rearrange("p (a c) -> p a c", c=cn))
                    hi = (dAH[:, m * cn:(m + n2) * cn]
                          .rearrange("p (a c) -> p a c", c=cn))
                    nc.vector.tensor_tensor(hi, lo, em, OP.mult)
                    m += n2
                for si in range(DA_ACT_FROM, S):
                    nc.scalar.activation(dAH[:, si * cn:(si + 1) * cn],
                                         dtn[:, :cn], AF.Exp,
                                         scale=float(si + 1))
                dA4 = dAH[:, :nb].rearrange("p (s t k) -> p s t k", s=S, k=K)
                nc.vector.memset(dA4[:, :, :, 0:1], 0.0)
                Bt = bigB.tile([CH, NB], f16, tag="Bt")
                nc.sync.dma_start(
                    Bt[:, :nb].rearrange("p (s c) -> p s c", s=S),
                    bcrow_dram[d][0:16, c0:c0 + cn].partition_broadcast(CH))
                u = bigU.tile([CH, NB], f16, tag="uC")
                nc.vector.tensor_tensor(
                    u[:, :nb].rearrange("p (s c) -> p s c", c=cn),
                    Bt[:, :nb].rearrange("p (s c) -> p s c", c=cn),
                    v[:, :cn].unsqueeze(1).broadcast_to([CH, S, cn]),
                    OP.mult)
                nc.vector.tensor_tensor_scan(dAH[:, :nb], dAH[:, :nb],
                                             u[:, :nb], 0.0, OP.mult, OP.add)
                Ct = bigU.tile([CH, NB], f16, tag="uC")
                nc.sync.dma_start(
                    Ct[:, :nb].rearrange("p (s c) -> p s c", s=S),
                    bcrow_dram[d][16:32, c0:c0 + cn].partition_broadcast(CH))
                nc.vector.tensor_tensor(dAH[:, :nb], dAH[:, :nb], Ct[:, :nb],
                                        OP.mult)
                h = S * cn
                while h > cn:
                    h2 = h // 2
                    nc.vector.tensor_tensor(dAH[:, 0:h2], dAH[:, 0:h2],
                                            dAH[:, h2:h], OP.add)
                    h = h2
                t1 = sb1.tile([CH, TC * K], f16, tag="t1_" + d)
                nc.vector.scalar_tensor_tensor(t1[:, :cn], xp[:, :cn],
                                               wt[d + '_dp'][:, 0:1],
                                               dAH[:, 0:cn], OP.mult, OP.add)
                yg = sb1.tile([CH, TC * K], f16, tag="yg_" + d)
                nc.vector.tensor_mul(yg[:, :cn], t1[:, :cn], zs[:, :cn])
                if fwd:
                    yg_f = yg
            o_ps = pso.tile([CH, 2, 512], f32, tag="ops")
            ygb3 = yg[:, :cn].rearrange("p (t k) -> p t k", k=K)
            for j, (ta, tb) in enumerate(tsubs):
                a, w = ta * K, (tb - ta) * K
                op3 = o_ps[:, j, :w].rearrange("p (t k) -> p t k", k=K)
                nc.tensor.matmul(op3, wt['bias_comb'][0:1, :],
                                 ones_r[0:1, 0:w]
                                 .rearrange("p (t k) -> p t k", k=K),
                                 start=True, stop=False)
                nc.tensor.matmul(o_ps[:, j, :w], wt['f_woc'][:],
                                 yg_f[:, a:a + w], start=False, stop=False)
                nc.tensor.matmul(op3, wt['b_woc'][:],
                                 ygb3[:, ta:tb, ::-1], start=False, stop=False)
                nc.tensor.matmul(o_ps[:, j, :w], wt['ident'][:],
                                 xbf[:, a:a + w], start=False, stop=True)
            o_sb = sb1.tile([CH, TC * K], f32, tag="osb")
            for j, (ta, tb) in enumerate(tsubs):
                a, w = ta * K, (tb - ta) * K
                nc.scalar.copy(o_sb[:, a:a + w], o_ps[:, j, :w])
            nc.sync.dma_start(o_d[:, c0:c0 + cn], o_sb[:, :cn])

    nc.compile()
    return nc


def kernel(**inputs):
    from concourse.bass_utils import run_bass_kernel_spmd

    if 'nc' not in _CACHE:
        _CACHE['nc'] = _build()
    nc = _CACHE['nc']
    W = _host_prep(inputs)
    x = np.asarray(inputs['x'], dtype=np.float32)  # (8, 128, 300, 30)
    in_maps = []
    for i in range(NCORES):
        m = dict(W)
        m['x'] = np.ascontiguousarray(x[i].reshape(CH, TKF))
        in_maps.append(m)
    res = run_bass_kernel_spmd(nc, in_maps, core_ids=list(range(NCORES)))
    out = np.stack([np.asarray(res.results[i]['out'], dtype=np.float32)
                    .reshape(CH, T, K) for i in range(NCORES)])
    return out
